# revision 56
# baseline (speedup 1.0000x reference)
"""GroupedQueryAttention (B=1, S=4096, D=1024, G=16 heads, DH=64) on 8 TRN2 NeuronCores.

Sharding: tensor-parallel over heads. Core c computes heads {2c, 2c+1}:
  - Q/K/V projections with column-sliced weights (128 out-dims per core),
    producing Q^T/K^T/V^T in [dout, seq] layout (host pre-transposes inputs).
    Inputs stream in as 512 KB half-row chunks on two HWDGE queues (sync for
    K/V, scalar for Q) so the load runs near HBM line rate. All attention-path
    matmuls are bf16 (fp32 PSUM accumulation); bias adds run on VectorE.
  - Flash-style attention without max-subtraction (scores are tiny:
    |s/8| < ~3). The two heads' QK^T matmuls are row-packed (K=64 each,
    tile_position rows 0-63 / 64-127) so they run concurrently on the PE.
  - exp on ScalarE over two PSUM banks at once (N=1024) with fused 1/8
    scale. The attention mask is folded multiplicatively into V-natural
    (V rows and the appended ones-column are zeroed for masked keys), so
    exp needs no per-chunk bias and softmax stays exact for any 0/1 mask.
  - Softmax denominator comes free via the ones-column appended to V in the
    PV matmul (PSUM row 64 accumulates sum_k exp).
  - Output projection with row-sliced Wo produces a partial (4096, 1024)
    output per core; each q-tile's output projection is deferred into the
    next q-tile's QK/PV stream so it fills PE slack instead of stalling
    ScalarE at tile boundaries. Host sums the 8 partials and adds bo.
"""

import os
import sys

for _p in ("/opt/trn_rl_repo", "/root/.axon_site/_ro/trn_rl_repo"):
    if os.path.isdir(_p) and _p not in sys.path:
        sys.path.insert(0, _p)

from contextlib import ExitStack

import numpy as np
import ml_dtypes

import concourse.bass as bass
import concourse.mybir as mybir
import concourse.tile as tile
from concourse import bacc
from concourse.bass_utils import run_bass_kernel_spmd
from concourse.masks import make_identity

S = 4096          # sequence length
D = 1024          # model dim
G = 16            # heads
DH = 64           # head dim
P = 128           # partitions
QT = 512          # q-tile (moving free dim)
KC = 128          # k-chunk
NCORES = 8
HPC = G // NCORES             # heads per core = 2
N_ST = S // QT                # 8 s-tiles of 512
N_KCH = D // P                # 8 contraction chunks for projections
N_KC = S // KC                # 32 k-chunks for attention
DSL = P                       # per-core dout slice (2 heads * 64)
XH = S // 2                   # input DMA chunk width (half row)

F32 = mybir.dt.float32
BF16 = mybir.dt.bfloat16
BF16_NP = ml_dtypes.bfloat16

_CACHE = {}


def _build_nc(dbg=False):
    key = ("nc", dbg)
    if key in _CACHE:
        return _CACHE[key]

    nc = bacc.Bacc(
        "TRN2", target_bir_lowering=False, debug=False, num_devices=NCORES
    )

    xqT = nc.dram_tensor("xqT", [D, S], BF16, kind="ExternalInput").ap()
    xkT = nc.dram_tensor("xkT", [D, S], BF16, kind="ExternalInput").ap()
    xvT = nc.dram_tensor("xvT", [D, S], BF16, kind="ExternalInput").ap()
    wqT = nc.dram_tensor("wqT", [P, N_KCH * DSL], BF16, kind="ExternalInput").ap()
    wkT = nc.dram_tensor("wkT", [P, N_KCH * DSL], BF16, kind="ExternalInput").ap()
    wvT = nc.dram_tensor("wvT", [P, N_KCH * DSL], BF16, kind="ExternalInput").ap()
    woT = nc.dram_tensor("woT", [DSL, D], BF16, kind="ExternalInput").ap()
    bq = nc.dram_tensor("bq", [DSL, 1], F32, kind="ExternalInput").ap()
    bk = nc.dram_tensor("bk", [DSL, 1], F32, kind="ExternalInput").ap()
    bv = nc.dram_tensor("bv", [DSL, 1], F32, kind="ExternalInput").ap()
    # multiplicative 0/1 mask, [key-in-chunk, chunk]
    mmul = nc.dram_tensor("mmul", [P, N_KC], F32, kind="ExternalInput").ap()
    out_d = nc.dram_tensor("out", [S, D], F32, kind="ExternalOutput").ap()
    if dbg:
        dbg_d = {
            n: nc.dram_tensor(f"dbg_{n}", shp, dt, kind="ExternalOutput").ap()
            for n, shp, dt in (
                ("qts", [P, S], BF16), ("kts", [P, S], BF16),
                ("vts", [P, S], F32),
                ("vn0", [P, N_KC * (DH + 4)], BF16),
                ("vn1", [P, N_KC * (DH + 4)], BF16),
                ("attnT", [P, S], BF16),
            )
        }

    with tile.TileContext(nc) as tc, ExitStack() as ctx:
        consts = ctx.enter_context(tc.tile_pool(name="consts", bufs=1))
        big = ctx.enter_context(tc.tile_pool(name="big", bufs=1))
        xkv_p = ctx.enter_context(tc.tile_pool(name="xkv", bufs=12))
        xq_p = ctx.enter_context(tc.tile_pool(name="xq", bufs=16))
        et_pool = ctx.enter_context(tc.tile_pool(name="et", bufs=6))
        small = ctx.enter_context(tc.tile_pool(name="small", bufs=2))
        oevict = ctx.enter_context(tc.tile_pool(name="oevict", bufs=4))
        # single 3-deep rotation of 2-bank PSUM slots serves projections,
        # V transposes, QK scores, and the output projection (6 banks);
        # the remaining 2 banks hold the two heads' PV accumulators.
        ps_qk = ctx.enter_context(tc.tile_pool(name="ps_qk", bufs=3, space="PSUM"))
        ps_pv = ctx.enter_context(tc.tile_pool(name="ps_pv", bufs=2, space="PSUM"))

        # ---- constants ----
        ident = consts.tile([P, P], F32)
        make_identity(nc, ident[:])

        # weights/biases load via the gpsimd SWDGE queue so they don't
        # head-of-line block the input streams on the two HWDGE queues;
        # tiny biases/mask first since projection evicts gate on them
        b_s = {}
        for name, bd in (("q", bq), ("k", bk), ("v", bv)):
            b = consts.tile([DSL, 1], F32, tag=f"b{name}")
            nc.gpsimd.dma_start(b[:], bd)
            b_s[name] = b
        mm_s = consts.tile([P, N_KC], F32, tag="mm")
        nc.gpsimd.dma_start(mm_s[:], mmul)
        w_s = {}
        for name, wd in (("k", wkT), ("q", wqT), ("v", wvT)):
            w = consts.tile([P, N_KCH * DSL], BF16, tag=f"w{name}")
            nc.gpsimd.dma_start(w[:], wd)
            w_s[name] = w
        wo_s = consts.tile([DSL, D], BF16, tag="wo")
        nc.gpsimd.dma_start(wo_s[:], woT)

        # ---- resident activations ----
        QTs = big.tile([P, S], BF16, tag="QTs")      # Q^T  [dout, s]
        KTs = big.tile([P, S], BF16, tag="KTs")      # K^T  [dout, s]
        VTs = big.tile([P, S], F32, tag="VTs")       # V^T  [dout, s]
        # V natural per head: [k-part, chunk, DH+1]; col DH is the mask
        # column (1/0) that makes PV also accumulate sum_k exp (softmax
        # denom) restricted to unmasked keys.
        Vnat = [
            big.tile([P, N_KC, DH + 4], BF16, tag=f"Vn{h}", name=f"Vnat{h}")
            for h in range(HPC)
        ]
        attnT = big.tile([P, S], BF16, tag="attnT")  # normalized attn^T [din, s]

        for h in range(HPC):
            # mask column: exactly the 0/1 mask per key
            nc.vector.tensor_copy(Vnat[h][:, :, DH], mm_s[:])

        # ---- phase 1: input DMA + projections ----
        # V first (split across both HWDGE queues) so PV never waits; then
        # K on sync || Q on scalar. 512 KB half-row chunks hit near HBM
        # line rate. Only K/Q half 0 projections run before attention; the
        # half-1 projections are interleaved into early attention steps so
        # the PE never idles waiting for their DMAs.
        def load_half(name, xd, half, engines, pool):
            hsl = slice(half * XH, (half + 1) * XH)
            xt = []
            for kc in range(N_KCH):
                x = pool.tile([P, XH], BF16, tag="xf", name=f"x{name}{kc}")
                engines[kc % len(engines)].dma_start(
                    x[:], xd[kc * P:(kc + 1) * P, hsl]
                )
                xt.append(x)
            return xt

        def proj_stile(name, xt, half, j, evict):
            st = half * (XH // QT) + j
            sl = slice(st * QT, (st + 1) * QT)
            jsl = slice(j * QT, (j + 1) * QT)
            ps = ps_qk.tile([P, QT], F32, tag="qk", name="ps")
            for kc in range(N_KCH):
                nc.tensor.matmul(
                    ps[:],
                    w_s[name][:, kc * DSL:(kc + 1) * DSL],
                    xt[kc][:, jsl],
                    start=(kc == 0),
                    stop=(kc == N_KCH - 1),
                )
            evict(ps, sl, st)

        def evict_k(ps, sl, st):
            nc.vector.tensor_scalar_add(KTs[:, sl], ps[:], b_s["k"][:])

        def evict_q(ps, sl, st):
            nc.vector.tensor_scalar_add(QTs[:, sl], ps[:], b_s["q"][:])

        def evict_v(ps, sl, st):
            nc.vector.tensor_scalar_add(VTs[:, sl], ps[:], b_s["v"][:])
            # transpose this s-tile of V^T into V natural (4 k-chunks),
            # applying the multiplicative key mask
            for h in range(HPC):
                hs = slice(h * DH, (h + 1) * DH)
                for jj in range(QT // KC):
                    kc = st * (QT // KC) + jj
                    pt = ps_qk.tile([P, DH], F32, tag="qk", name="pt")
                    nc.tensor.transpose(
                        pt[:], VTs[hs, kc * KC:(kc + 1) * KC], ident[hs, hs]
                    )
                    nc.vector.tensor_mul(
                        Vnat[h][:, kc, 0:DH],
                        pt[:],
                        mm_s[:, kc:kc + 1].to_broadcast((P, DH)),
                    )

        # DMA issue order per queue — sync: K h0, V h0, V h1;
        # scalar: Q h0, K h1, Q h1. The first QK needs K h0 + Q h0 (~23us),
        # V h0 gates only the up-front V projection; K h1 and V h1 land in
        # time for the deferred drains during q-tile 0.
        xk = [load_half("k", xkT, 0, [nc.sync], xkv_p)]
        xq = [load_half("q", xqT, 0, [nc.scalar], xq_p)]
        # V half 0 rides the otherwise-idle gpsimd SWDGE queue so it lands
        # in parallel with K/Q half 0 on the two HWDGE queues
        xv = [load_half("v", xvT, 0, [nc.gpsimd], xkv_p)]
        xv.append(load_half("v", xvT, 1, [nc.sync, nc.scalar], xkv_p))
        xk.append(load_half("k", xkT, 1, [nc.scalar], xq_p))
        xq.append(load_half("q", xqT, 1, [nc.scalar], xq_p))

        for j in range(XH // QT):
            proj_stile("k", xk[0], 0, j, evict_k)
        for j in range(XH // QT):
            proj_stile("q", xq[0], 0, j, evict_q)
        for j in range(XH // QT):
            proj_stile("v", xv[0], 0, j, evict_v)

        # Deferred half-1 work, split into ~1us items drained one per
        # attention step so the ScalarE exp stream never stalls on a long
        # PE block. Order interleaves K (QK needs chunk 16 at step 16)
        # with V (PV needs chunk 16 at step 17); Q half 1 drains in qt 1.
        def make_proj_parts(name, xt, j, evict):
            st = (XH // QT) + j
            sl = slice(st * QT, (st + 1) * QT)
            jsl = slice(j * QT, (j + 1) * QT)
            cell = {}

            def part0():
                ps = ps_qk.tile([P, QT], F32, tag="qk", name="ps")
                cell["ps"] = ps
                for kc in range(N_KCH // 2):
                    nc.tensor.matmul(
                        ps[:], w_s[name][:, kc * DSL:(kc + 1) * DSL],
                        xt[kc][:, jsl], start=(kc == 0), stop=False,
                    )

            def part1():
                ps = cell["ps"]
                for kc in range(N_KCH // 2, N_KCH):
                    nc.tensor.matmul(
                        ps[:], w_s[name][:, kc * DSL:(kc + 1) * DSL],
                        xt[kc][:, jsl], start=False, stop=(kc == N_KCH - 1),
                    )
                evict(ps, sl, st)

            return [part0, part1]

        def v_transpose_item(st, h, jj0=0, njj=QT // KC):
            hs = slice(h * DH, (h + 1) * DH)
            for jj in range(jj0, jj0 + njj):
                kc = st * (QT // KC) + jj
                pt = ps_qk.tile([P, DH], F32, tag="qk", name="pt")
                nc.tensor.transpose(
                    pt[:], VTs[hs, kc * KC:(kc + 1) * KC], ident[hs, hs]
                )
                nc.vector.tensor_mul(
                    Vnat[h][:, kc, 0:DH],
                    pt[:],
                    mm_s[:, kc:kc + 1].to_broadcast((P, DH)),
                )

        def evict_v_bias(ps, sl, st):
            nc.vector.tensor_scalar_add(VTs[:, sl], ps[:], b_s["v"][:])

        deferred = []
        for j in range(XH // QT):
            for p in make_proj_parts("k", xk[1], j, evict_k):
                deferred.append((0, p))
            for p in make_proj_parts("v", xv[1], j, evict_v_bias):
                deferred.append((0, p))
            st = (XH // QT) + j
            for h in range(HPC):
                for jj0 in (0, 2):
                    deferred.append((0, lambda st=st, h=h, jj0=jj0:
                                     v_transpose_item(st, h, jj0, 2)))
        for j in range(XH // QT):
            for p in make_proj_parts("q", xq[1], j, evict_q):
                deferred.append((1, p))

        pending = [None]

        # ---- phase 2+3: attention, software-pipelined ----
        def outproj_block(qt, blk):
            st = qt * (QT // P) + blk // 2
            nt = blk % 2
            po = ps_qk.tile([P, QT], F32, tag="qk", name="po")
            nc.tensor.matmul(
                po[:],
                attnT[:, st * P:(st + 1) * P],
                wo_s[:, nt * QT:(nt + 1) * QT],
                start=True, stop=True,
            )
            ot = oevict.tile([P, QT], F32, tag="ot")
            nc.vector.tensor_copy(ot[:], po[:])
            eng = nc.sync if nt == 0 else nc.scalar
            eng.dma_start(
                out_d[st * P:(st + 1) * P, nt * QT:(nt + 1) * QT], ot[:]
            )

        def emit_pv(pv, ets, kc):
            et = ets.pop(kc)
            for h in range(HPC):
                nc.tensor.matmul(
                    pv[h][:],
                    Vnat[h][:, kc, 0:DH + 1],
                    et[:, h * QT:(h + 1) * QT],
                    start=(kc == 0), stop=(kc == N_KC - 1),
                )

        for qt in range(N_ST):
            qsl = slice(qt * QT, (qt + 1) * QT)
            pv = [
                ps_pv.tile([DH + 1, QT], F32, tag="pv", name=f"pv{h}")
                for h in range(HPC)
            ]
            ets = {}
            for kc in range(N_KC):
                # both heads' QK^T into one 2-bank tile, adjacent emission
                # so the K=64 matmuls row-pack and run concurrently
                sc = ps_qk.tile([P, 2 * QT], F32, tag="qk", name="sc")
                for h in range(HPC):
                    hs = slice(h * DH, (h + 1) * DH)
                    nc.tensor.matmul(
                        sc[:, h * QT:(h + 1) * QT],
                        KTs[hs, kc * KC:(kc + 1) * KC],
                        QTs[hs, qsl],
                        start=True, stop=True,
                    )
                et = et_pool.tile([P, 2 * QT], BF16, tag="et")
                nc.scalar.activation(
                    et[:], sc[:],
                    mybir.ActivationFunctionType.Exp,
                    scale=0.125,
                )
                ets[kc] = et
                # deferred half-1 projection items ride in PE slack,
                # emitted right after the exp so the next step's QK isn't
                # pushed behind them; in qt 0 they wait until their DMAs
                # have had time to land
                ndrain = 2 if (qt == 0 and kc >= 5) else (1 if qt > 0 else 0)
                for _ in range(ndrain):
                    if deferred and deferred[0][0] <= qt:
                        deferred.pop(0)[1]()
                # the previous q-tile's last PVs + normalization ride here,
                # after this tile's first QK/exp, so the exp stream never
                # waits for them at the boundary
                if kc == 1 and pending[0] is not None:
                    pending[0]()
                    pending[0] = None
                # PV lags two steps: its exp is long done, PE never stalls
                # on ScalarE
                if kc > 1:
                    emit_pv(pv, ets, kc - 2)
                if qt > 0 and 9 <= kc < 33 and kc % 3 == 0:
                    outproj_block(qt - 1, (kc - 9) // 3)

            def finalize(pv=pv, ets=ets, qsl=qsl):
                # normalize: attnT[h, qsl] = pv[0:DH] * (1/pv[DH]).
                # Both pv reads (numerator and denominator copies) happen
                # immediately, so the pv PSUM banks hand off to the next
                # q-tile without waiting for the reciprocal chain.
                emit_pv(pv, ets, N_KC - 2)
                emit_pv(pv, ets, N_KC - 1)
                for h in range(HPC):
                    hs = slice(h * DH, (h + 1) * DH)
                    pnum = small.tile([DH, QT], F32, tag="pnum")
                    nc.vector.tensor_copy(pnum[:], pv[h][0:DH, :])
                    den = small.tile([1, QT], F32, tag="den")
                    nc.vector.tensor_copy(den[:], pv[h][DH:DH + 1, :])
                    rec = small.tile([1, QT], F32, tag="rec")
                    nc.vector.reciprocal_approx_fast(rec[:], den[:])
                    bc = small.tile([DH, QT], F32, tag="bc")
                    nc.gpsimd.partition_broadcast(bc[:], rec[:])
                    nc.vector.tensor_mul(attnT[hs, qsl], pnum[:], bc[:])

            pending[0] = finalize
        pending[0]()
        pending[0] = None
        for blk in range(8):
            outproj_block(N_ST - 1, blk)

        if dbg:
            for name, t in (("qts", QTs), ("kts", KTs), ("vts", VTs),
                            ("attnT", attnT)):
                nc.sync.dma_start(dbg_d[name][:, :], t[:])
            nc.sync.dma_start(dbg_d["vn0"][:, :], Vnat[0][:])
            nc.sync.dma_start(dbg_d["vn1"][:, :], Vnat[1][:])

    nc.compile()
    _CACHE[key] = nc
    return nc


def _prep_in_maps(query, key, value, mask, Wq, bq, Wk, bk, Wv, bv, Wo, bo):
    f = np.float32
    qT = np.asarray(query, dtype=f)[0].T.astype(BF16_NP)
    kT = np.asarray(key, dtype=f)[0].T.astype(BF16_NP)
    vT = np.asarray(value, dtype=f)[0].T.astype(BF16_NP)
    mm = (np.asarray(mask)[0] != 0).astype(f)
    mm = np.ascontiguousarray(mm.reshape(N_KC, KC).T)  # [128, 32]
    WqT, WkT, WvT, WoT = (np.asarray(W, dtype=f).T.astype(BF16_NP)
                          for W in (Wq, Wk, Wv, Wo))
    in_maps = []
    for c in range(NCORES):
        cs = slice(c * DSL, (c + 1) * DSL)
        def wlay(WT):
            # [D_in, DSL] slice -> [P, N_KCH*DSL]: contraction chunks side
            # by side so one DMA fills the whole stationary-weight tile
            blocks = [WT[kc * P:(kc + 1) * P, cs] for kc in range(N_KCH)]
            return np.ascontiguousarray(np.concatenate(blocks, axis=1))

        in_maps.append({
            "xqT": qT, "xkT": kT, "xvT": vT,
            "wqT": wlay(WqT),
            "wkT": wlay(WkT),
            "wvT": wlay(WvT),
            "woT": np.ascontiguousarray(WoT[cs, :]),
            "bq": np.ascontiguousarray(bq[cs].astype(f, copy=False)).reshape(DSL, 1),
            "bk": np.ascontiguousarray(bk[cs].astype(f, copy=False)).reshape(DSL, 1),
            "bv": np.ascontiguousarray(bv[cs].astype(f, copy=False)).reshape(DSL, 1),
            "mmul": mm,
        })
    return in_maps


def run(inputs, trace=False, trace_kwargs=None, dbg=False):
    nc = _build_nc(dbg=dbg)
    in_maps = _prep_in_maps(**inputs)
    res = run_bass_kernel_spmd(
        nc, in_maps, core_ids=list(range(NCORES)), trace=trace,
        **(trace_kwargs or {}),
    )
    bo = np.asarray(inputs["bo"], dtype=np.float32)
    acc = np.zeros((S, D), dtype=np.float32)
    for r in res.results:
        acc += r["out"]
    out = (acc + bo[None, :]).astype(np.float32)[None]
    return out, res


def kernel(**inputs):
    out, _ = run(inputs, trace=False)
    return out


# revision 60
# speedup vs baseline: 1.0005x; 1.0005x over previous
"""GroupedQueryAttention (B=1, S=4096, D=1024, G=16 heads, DH=64) on 8 TRN2 NeuronCores.

Sharding: tensor-parallel over heads. Core c computes heads {2c, 2c+1}:
  - Q/K/V projections with column-sliced weights (128 out-dims per core),
    producing Q^T/K^T/V^T in [dout, seq] layout (host pre-transposes inputs).
    Inputs stream in as 512 KB half-row chunks on two HWDGE queues (sync for
    K/V, scalar for Q) so the load runs near HBM line rate. All attention-path
    matmuls are bf16 (fp32 PSUM accumulation); bias adds run on VectorE.
  - Flash-style attention without max-subtraction (scores are tiny:
    |s/8| < ~3). The two heads' QK^T matmuls are row-packed (K=64 each,
    tile_position rows 0-63 / 64-127) so they run concurrently on the PE.
  - exp on ScalarE over two PSUM banks at once (N=1024) with fused 1/8
    scale. The attention mask is folded multiplicatively into V-natural
    (V rows and the appended ones-column are zeroed for masked keys), so
    exp needs no per-chunk bias and softmax stays exact for any 0/1 mask.
  - Softmax denominator comes free via the ones-column appended to V in the
    PV matmul (PSUM row 64 accumulates sum_k exp).
  - Output projection with row-sliced Wo produces a partial (4096, 1024)
    output per core; each q-tile's output projection is deferred into the
    next q-tile's QK/PV stream so it fills PE slack instead of stalling
    ScalarE at tile boundaries. Host sums the 8 partials and adds bo.
"""

import os
import sys

for _p in ("/opt/trn_rl_repo", "/root/.axon_site/_ro/trn_rl_repo"):
    if os.path.isdir(_p) and _p not in sys.path:
        sys.path.insert(0, _p)

from contextlib import ExitStack

import numpy as np
import ml_dtypes

import concourse.bass as bass
import concourse.mybir as mybir
import concourse.tile as tile
from concourse import bacc
from concourse.bass_utils import run_bass_kernel_spmd
from concourse.masks import make_identity

S = 4096          # sequence length
D = 1024          # model dim
G = 16            # heads
DH = 64           # head dim
P = 128           # partitions
QT = 512          # q-tile (moving free dim)
KC = 128          # k-chunk
NCORES = 8
HPC = G // NCORES             # heads per core = 2
N_ST = S // QT                # 8 s-tiles of 512
N_KCH = D // P                # 8 contraction chunks for projections
N_KC = S // KC                # 32 k-chunks for attention
DSL = P                       # per-core dout slice (2 heads * 64)
XH = S // 2                   # input DMA chunk width (half row)

F32 = mybir.dt.float32
BF16 = mybir.dt.bfloat16
BF16_NP = ml_dtypes.bfloat16

_CACHE = {}


def _build_nc(dbg=False):
    key = ("nc", dbg)
    if key in _CACHE:
        return _CACHE[key]

    nc = bacc.Bacc(
        "TRN2", target_bir_lowering=False, debug=False, num_devices=NCORES
    )

    xqT = nc.dram_tensor("xqT", [D, S], BF16, kind="ExternalInput").ap()
    xkT = nc.dram_tensor("xkT", [D, S], BF16, kind="ExternalInput").ap()
    xvT = nc.dram_tensor("xvT", [D, S], BF16, kind="ExternalInput").ap()
    wqT = nc.dram_tensor("wqT", [N_KCH, P, DSL], BF16, kind="ExternalInput").ap()
    wkT = nc.dram_tensor("wkT", [N_KCH, P, DSL], BF16, kind="ExternalInput").ap()
    wvT = nc.dram_tensor("wvT", [N_KCH, P, DSL], BF16, kind="ExternalInput").ap()
    woT = nc.dram_tensor("woT", [DSL, D], BF16, kind="ExternalInput").ap()
    bq = nc.dram_tensor("bq", [DSL, 1], F32, kind="ExternalInput").ap()
    bk = nc.dram_tensor("bk", [DSL, 1], F32, kind="ExternalInput").ap()
    bv = nc.dram_tensor("bv", [DSL, 1], F32, kind="ExternalInput").ap()
    # multiplicative 0/1 mask, [key-in-chunk, chunk]
    mmul = nc.dram_tensor("mmul", [P, N_KC], F32, kind="ExternalInput").ap()
    out_d = nc.dram_tensor("out", [S, D], F32, kind="ExternalOutput").ap()
    if dbg:
        dbg_d = {
            n: nc.dram_tensor(f"dbg_{n}", shp, dt, kind="ExternalOutput").ap()
            for n, shp, dt in (
                ("qts", [P, S], BF16), ("kts", [P, S], BF16),
                ("vts", [P, S], F32),
                ("vn0", [P, N_KC * (DH + 4)], BF16),
                ("vn1", [P, N_KC * (DH + 4)], BF16),
                ("attnT", [P, S], BF16),
            )
        }

    with tile.TileContext(nc) as tc, ExitStack() as ctx:
        consts = ctx.enter_context(tc.tile_pool(name="consts", bufs=1))
        big = ctx.enter_context(tc.tile_pool(name="big", bufs=1))
        xkv_p = ctx.enter_context(tc.tile_pool(name="xkv", bufs=12))
        xq_p = ctx.enter_context(tc.tile_pool(name="xq", bufs=16))
        et_pool = ctx.enter_context(tc.tile_pool(name="et", bufs=8))
        small = ctx.enter_context(tc.tile_pool(name="small", bufs=2))
        oevict = ctx.enter_context(tc.tile_pool(name="oevict", bufs=3))
        # single 3-deep rotation of 2-bank PSUM slots serves projections,
        # V transposes, QK scores, and the output projection (6 banks);
        # the remaining 2 banks hold the two heads' PV accumulators.
        ps_qk = ctx.enter_context(tc.tile_pool(name="ps_qk", bufs=3, space="PSUM"))
        ps_pv = ctx.enter_context(tc.tile_pool(name="ps_pv", bufs=2, space="PSUM"))

        # ---- constants ----
        ident = consts.tile([P, P], F32)
        make_identity(nc, ident[:])

        # weights/biases load via the gpsimd SWDGE queue so they don't
        # head-of-line block the input streams on the two HWDGE queues;
        # tiny biases/mask first since projection evicts gate on them
        b_s = {}
        for name, bd in (("q", bq), ("k", bk), ("v", bv)):
            b = consts.tile([DSL, 1], F32, tag=f"b{name}")
            nc.gpsimd.dma_start(b[:], bd)
            b_s[name] = b
        mm_s = consts.tile([P, N_KC], F32, tag="mm")
        nc.gpsimd.dma_start(mm_s[:], mmul)
        w_s = {}
        for name, wd in (("k", wkT), ("q", wqT), ("v", wvT)):
            w = consts.tile([P, N_KCH * DSL], BF16, tag=f"w{name}")
            for kc in range(N_KCH):
                nc.gpsimd.dma_start(w[:, kc * DSL:(kc + 1) * DSL], wd[kc])
            w_s[name] = w
        wo_s = consts.tile([DSL, D], BF16, tag="wo")
        nc.gpsimd.dma_start(wo_s[:], woT)

        # ---- resident activations ----
        QTs = big.tile([P, S], BF16, tag="QTs")      # Q^T  [dout, s]
        KTs = big.tile([P, S], BF16, tag="KTs")      # K^T  [dout, s]
        VTs = big.tile([P, S], F32, tag="VTs")       # V^T  [dout, s]
        # V natural per head: [k-part, chunk, DH+1]; col DH is the mask
        # column (1/0) that makes PV also accumulate sum_k exp (softmax
        # denom) restricted to unmasked keys.
        Vnat = [
            big.tile([P, N_KC, DH + 4], BF16, tag=f"Vn{h}", name=f"Vnat{h}")
            for h in range(HPC)
        ]
        attnT = big.tile([P, S], BF16, tag="attnT")  # normalized attn^T [din, s]

        for h in range(HPC):
            # mask column: exactly the 0/1 mask per key
            nc.vector.tensor_copy(Vnat[h][:, :, DH], mm_s[:])

        # ---- phase 1: input DMA + projections ----
        # V first (split across both HWDGE queues) so PV never waits; then
        # K on sync || Q on scalar. 512 KB half-row chunks hit near HBM
        # line rate. Only K/Q half 0 projections run before attention; the
        # half-1 projections are interleaved into early attention steps so
        # the PE never idles waiting for their DMAs.
        def load_half(name, xd, half, engines, pool):
            hsl = slice(half * XH, (half + 1) * XH)
            xt = []
            for kc in range(N_KCH):
                x = pool.tile([P, XH], BF16, tag="xf", name=f"x{name}{kc}")
                engines[kc % len(engines)].dma_start(
                    x[:], xd[kc * P:(kc + 1) * P, hsl]
                )
                xt.append(x)
            return xt

        def proj_stile(name, xt, half, j, evict):
            st = half * (XH // QT) + j
            sl = slice(st * QT, (st + 1) * QT)
            jsl = slice(j * QT, (j + 1) * QT)
            ps = ps_qk.tile([P, QT], F32, tag="qk", name="ps")
            for kc in range(N_KCH):
                nc.tensor.matmul(
                    ps[:],
                    w_s[name][:, kc * DSL:(kc + 1) * DSL],
                    xt[kc][:, jsl],
                    start=(kc == 0),
                    stop=(kc == N_KCH - 1),
                )
            evict(ps, sl, st)

        def evict_k(ps, sl, st):
            nc.vector.tensor_scalar_add(KTs[:, sl], ps[:], b_s["k"][:])

        def evict_q(ps, sl, st):
            nc.vector.tensor_scalar_add(QTs[:, sl], ps[:], b_s["q"][:])

        def evict_v_bias(ps, sl, st):
            nc.vector.tensor_scalar_add(VTs[:, sl], ps[:], b_s["v"][:])

        def evict_v(ps, sl, st):
            nc.vector.tensor_scalar_add(VTs[:, sl], ps[:], b_s["v"][:])
            # transpose this s-tile of V^T into V natural (4 k-chunks),
            # applying the multiplicative key mask
            for h in range(HPC):
                hs = slice(h * DH, (h + 1) * DH)
                for jj in range(QT // KC):
                    kc = st * (QT // KC) + jj
                    pt = ps_qk.tile([P, DH], F32, tag="qk", name="pt")
                    nc.tensor.transpose(
                        pt[:], VTs[hs, kc * KC:(kc + 1) * KC], ident[hs, hs]
                    )
                    nc.vector.tensor_mul(
                        Vnat[h][:, kc, 0:DH],
                        pt[:],
                        mm_s[:, kc:kc + 1].to_broadcast((P, DH)),
                    )

        # DMA issue order per queue — sync: K h0, V h0, V h1;
        # scalar: Q h0, K h1, Q h1. The first QK needs K h0 + Q h0 (~23us),
        # V h0 gates only the up-front V projection; K h1 and V h1 land in
        # time for the deferred drains during q-tile 0.
        xk = [load_half("k", xkT, 0, [nc.sync], xkv_p)]
        xq = [load_half("q", xqT, 0, [nc.scalar], xq_p)]
        xv = [load_half("v", xvT, 0, [nc.sync, nc.scalar], xkv_p)]
        xv.append(load_half("v", xvT, 1, [nc.sync, nc.scalar], xkv_p))
        xk.append(load_half("k", xkT, 1, [nc.scalar], xq_p))
        xq.append(load_half("q", xqT, 1, [nc.scalar], xq_p))

        for j in range(XH // QT):
            proj_stile("k", xk[0], 0, j, evict_k)
        for j in range(XH // QT):
            proj_stile("q", xq[0], 0, j, evict_q)
        for j in range(XH // QT):
            proj_stile("v", xv[0], 0, j, evict_v_bias)

        # Deferred half-1 work, split into ~1us items drained one per
        # attention step so the ScalarE exp stream never stalls on a long
        # PE block. Order interleaves K (QK needs chunk 16 at step 16)
        # with V (PV needs chunk 16 at step 17); Q half 1 drains in qt 1.
        def make_proj_parts(name, xt, j, evict):
            st = (XH // QT) + j
            sl = slice(st * QT, (st + 1) * QT)
            jsl = slice(j * QT, (j + 1) * QT)
            cell = {}

            def part0():
                ps = ps_qk.tile([P, QT], F32, tag="qk", name="ps")
                cell["ps"] = ps
                for kc in range(N_KCH // 2):
                    nc.tensor.matmul(
                        ps[:], w_s[name][:, kc * DSL:(kc + 1) * DSL],
                        xt[kc][:, jsl], start=(kc == 0), stop=False,
                    )

            def part1():
                ps = cell["ps"]
                for kc in range(N_KCH // 2, N_KCH):
                    nc.tensor.matmul(
                        ps[:], w_s[name][:, kc * DSL:(kc + 1) * DSL],
                        xt[kc][:, jsl], start=False, stop=(kc == N_KCH - 1),
                    )
                evict(ps, sl, st)

            return [part0, part1]

        def v_transpose_item(st, h, jj0=0, njj=QT // KC):
            hs = slice(h * DH, (h + 1) * DH)
            for jj in range(jj0, jj0 + njj):
                kc = st * (QT // KC) + jj
                pt = ps_qk.tile([P, DH], F32, tag="qk", name="pt")
                nc.tensor.transpose(
                    pt[:], VTs[hs, kc * KC:(kc + 1) * KC], ident[hs, hs]
                )
                nc.vector.tensor_mul(
                    Vnat[h][:, kc, 0:DH],
                    pt[:],
                    mm_s[:, kc:kc + 1].to_broadcast((P, DH)),
                )

        def evict_v_bias(ps, sl, st):
            nc.vector.tensor_scalar_add(VTs[:, sl], ps[:], b_s["v"][:])

        deferred = []
        for st in range(XH // QT):  # V half 0 transposes: Vnat chunks 0-15
            for h in range(HPC):
                for jj0 in (0, 2):
                    deferred.append((0, lambda st=st, h=h, jj0=jj0:
                                     v_transpose_item(st, h, jj0, 2)))
        for j in range(XH // QT):
            for p in make_proj_parts("k", xk[1], j, evict_k):
                deferred.append((0, p))
            for p in make_proj_parts("v", xv[1], j, evict_v_bias):
                deferred.append((0, p))
            st = (XH // QT) + j
            for h in range(HPC):
                for jj0 in (0, 2):
                    deferred.append((0, lambda st=st, h=h, jj0=jj0:
                                     v_transpose_item(st, h, jj0, 2)))
        for j in range(XH // QT):
            for p in make_proj_parts("q", xq[1], j, evict_q):
                deferred.append((1, p))

        pending = [None]

        # ---- phase 2+3: attention, software-pipelined ----
        def outproj_block(qt, blk):
            st = qt * (QT // P) + blk // 2
            nt = blk % 2
            po = ps_qk.tile([P, QT], F32, tag="qk", name="po")
            nc.tensor.matmul(
                po[:],
                attnT[:, st * P:(st + 1) * P],
                wo_s[:, nt * QT:(nt + 1) * QT],
                start=True, stop=True,
            )
            ot = oevict.tile([P, QT], F32, tag="ot")
            nc.vector.tensor_copy(ot[:], po[:])
            eng = nc.sync if nt == 0 else nc.scalar
            eng.dma_start(
                out_d[st * P:(st + 1) * P, nt * QT:(nt + 1) * QT], ot[:]
            )

        def emit_pv(pv, ets, kc):
            et = ets.pop(kc)
            for h in range(HPC):
                nc.tensor.matmul(
                    pv[h][:],
                    Vnat[h][:, kc, 0:DH + 1],
                    et[:, h * QT:(h + 1) * QT],
                    start=(kc == 0), stop=(kc == N_KC - 1),
                )

        for qt in range(N_ST):
            qsl = slice(qt * QT, (qt + 1) * QT)
            pv = [
                ps_pv.tile([DH + 1, QT], F32, tag="pv", name=f"pv{h}")
                for h in range(HPC)
            ]
            ets = {}
            for kc in range(N_KC):
                # both heads' QK^T into one 2-bank tile, adjacent emission
                # so the K=64 matmuls row-pack and run concurrently
                sc = ps_qk.tile([P, 2 * QT], F32, tag="qk", name="sc")
                for h in range(HPC):
                    hs = slice(h * DH, (h + 1) * DH)
                    nc.tensor.matmul(
                        sc[:, h * QT:(h + 1) * QT],
                        KTs[hs, kc * KC:(kc + 1) * KC],
                        QTs[hs, qsl],
                        start=True, stop=True,
                    )
                et = et_pool.tile([P, 2 * QT], BF16, tag="et")
                nc.scalar.activation(
                    et[:], sc[:],
                    mybir.ActivationFunctionType.Exp,
                    scale=0.125,
                )
                ets[kc] = et
                # deferred half-1 projection items ride in PE slack,
                # emitted right after the exp so the next step's QK isn't
                # pushed behind them; in qt 0 they wait until their DMAs
                # have had time to land
                ndrain = 2 if qt == 0 else 1
                for _ in range(ndrain):
                    if deferred and deferred[0][0] <= qt:
                        deferred.pop(0)[1]()
                # the previous q-tile's last PVs + normalization ride here,
                # after this tile's first QK/exp, so the exp stream never
                # waits for them at the boundary
                if kc == 1 and pending[0] is not None:
                    pending[0]()
                    pending[0] = None
                # PV lags the exp stream (deep in qt 0 while V-natural
                # is still being built), PE never stalls on ScalarE
                lag = 6 if qt == 0 else 2
                if kc >= lag:
                    emit_pv(pv, ets, kc - lag)
                if qt > 0 and 9 <= kc < 33 and kc % 3 == 0:
                    outproj_block(qt - 1, (kc - 9) // 3)

            def finalize(pv=pv, ets=ets, qsl=qsl, lag=lag):
                # normalize: attnT[h, qsl] = pv[0:DH] * (1/pv[DH]).
                # Both pv reads (numerator and denominator copies) happen
                # immediately, so the pv PSUM banks hand off to the next
                # q-tile without waiting for the reciprocal chain.
                for k in range(N_KC - lag, N_KC):
                    emit_pv(pv, ets, k)
                for h in range(HPC):
                    hs = slice(h * DH, (h + 1) * DH)
                    pnum = small.tile([DH, QT], F32, tag="pnum")
                    nc.vector.tensor_copy(pnum[:], pv[h][0:DH, :])
                    den = small.tile([1, QT], F32, tag="den")
                    nc.vector.tensor_copy(den[:], pv[h][DH:DH + 1, :])
                    rec = small.tile([1, QT], F32, tag="rec")
                    nc.vector.reciprocal_approx_fast(rec[:], den[:])
                    bc = small.tile([DH, QT], F32, tag="bc")
                    nc.gpsimd.partition_broadcast(bc[:], rec[:])
                    nc.vector.tensor_mul(attnT[hs, qsl], pnum[:], bc[:])

            pending[0] = finalize
        pending[0]()
        pending[0] = None
        for blk in range(8):
            outproj_block(N_ST - 1, blk)

        if dbg:
            for name, t in (("qts", QTs), ("kts", KTs), ("vts", VTs),
                            ("attnT", attnT)):
                nc.sync.dma_start(dbg_d[name][:, :], t[:])
            nc.sync.dma_start(dbg_d["vn0"][:, :], Vnat[0][:])
            nc.sync.dma_start(dbg_d["vn1"][:, :], Vnat[1][:])

    nc.compile()
    _CACHE[key] = nc
    return nc


def _prep_in_maps(query, key, value, mask, Wq, bq, Wk, bk, Wv, bv, Wo, bo):
    f = np.float32
    qT = np.asarray(query, dtype=f)[0].T.astype(BF16_NP)
    kT = np.asarray(key, dtype=f)[0].T.astype(BF16_NP)
    vT = np.asarray(value, dtype=f)[0].T.astype(BF16_NP)
    mm = (np.asarray(mask)[0] != 0).astype(f)
    mm = np.ascontiguousarray(mm.reshape(N_KC, KC).T)  # [128, 32]
    WqT, WkT, WvT, WoT = (np.asarray(W, dtype=f).T.astype(BF16_NP)
                          for W in (Wq, Wk, Wv, Wo))
    in_maps = []
    for c in range(NCORES):
        cs = slice(c * DSL, (c + 1) * DSL)
        in_maps.append({
            "xqT": qT, "xkT": kT, "xvT": vT,
            "wqT": np.ascontiguousarray(WqT[:, cs]).reshape(N_KCH, P, DSL),
            "wkT": np.ascontiguousarray(WkT[:, cs]).reshape(N_KCH, P, DSL),
            "wvT": np.ascontiguousarray(WvT[:, cs]).reshape(N_KCH, P, DSL),
            "woT": np.ascontiguousarray(WoT[cs, :]),
            "bq": np.ascontiguousarray(bq[cs].astype(f, copy=False)).reshape(DSL, 1),
            "bk": np.ascontiguousarray(bk[cs].astype(f, copy=False)).reshape(DSL, 1),
            "bv": np.ascontiguousarray(bv[cs].astype(f, copy=False)).reshape(DSL, 1),
            "mmul": mm,
        })
    return in_maps


def run(inputs, trace=False, trace_kwargs=None, dbg=False):
    nc = _build_nc(dbg=dbg)
    in_maps = _prep_in_maps(**inputs)
    res = run_bass_kernel_spmd(
        nc, in_maps, core_ids=list(range(NCORES)), trace=trace,
        **(trace_kwargs or {}),
    )
    bo = np.asarray(inputs["bo"], dtype=np.float32)
    acc = np.zeros((S, D), dtype=np.float32)
    for r in res.results:
        acc += r["out"]
    out = (acc + bo[None, :]).astype(np.float32)[None]
    return out, res


def kernel(**inputs):
    out, _ = run(inputs, trace=False)
    return out
